# revision 71
# baseline (speedup 1.0000x reference)
"""LocallyConnected1d Trainium2 kernel (bf16, paired-position matmuls).

Problem: out[b, oc, w] = sum_{ic,k} xp[b, ic, w+k] * W[w, oc, ic, k] + bias[oc, w]
  x: (32, 64, 2048) f32, weights: (2048, 64, 64, 3) f32, bias: (64, 2048) f32
  out: (32, 64, 2048) f32.  xp = x padded by 1 on both sides of the last axis.

Sharding: output_width (2048) split into 8 chunks of 256, one per core.

Math per core: positions are processed in PAIRS (p, p+1).  Each pair needs the
6 tap-matrices W[p,:,:,0..2], W[p+1,:,:,0..2] contracted with x columns
p..p+3.  Two bf16 matmuls with FULL 128x128 stationary operands cover all six:

  rhs column tile xpp[:, c, :] stacks x cols (c, c+1) in 128 partitions
  (xpp[j, c, b] = xp[j%64, ws + c + j//64, b]).

  MM1 (rhs = xpp[:, p]):   stationary S1 blocks [row-block x-col, col-block pos]:
     (p+0 -> p): tap0   (p+1 -> p): tap1   (p+0 -> p+1): ZERO  (p+1 -> p+1): tap0
  MM2 (rhs = xpp[:, p+2]): stationary S2:
     (p+2 -> p): tap2   (p+3 -> p): ZERO   (p+2 -> p+1): tap1  (p+3 -> p+1): tap2

  PSUM out [128, 32]: partitions 0-63 = oc of pos p, 64-127 = oc of pos p+1.

fp32 matmuls cost 4 PE cycles/row and double LDWEIGHTS; bf16 costs 1
cycle/row and gets FWL (fast weight load) on the full-128-column contiguous
stationaries, cutting TensorE time ~6x vs the fp32 baseline (~10.5us/core).
The weight stream (8.4 MB/core incl. the two shipped zero blocks) rides the
sync HWDGE queue as back-to-back fully-contiguous 128-partition transfers
(~350-440 GB/s measured; 64-partition or strided-dest transfers measured
2-3x slower, which is why the zeros are shipped rather than skipped).  Bias
is folded in during PSUM evacuation on the Vector engine with a stride-0
(broadcast over batch) access pattern; outputs stage in SBUF and store in
two fat DMAs at the stream tail.  Total DMA ~11.6 MB/core vs ~21 MB fp32
baseline.  bf16 rounding keeps rel err ~3e-3, well under the 2e-2 gate.

Host-side prep (numpy): pair-major full-stationary relayout wfull
(slice-major flat), xpp (x twice, offset by one column), bias in pair
layout.  Casts to bf16 via ml_dtypes.
"""

import ml_dtypes
import numpy as np

import concourse.bacc as bacc
import concourse.mybir as mybir
import concourse.tile as tile
from concourse.bass import broadcast_tensor_aps
from concourse.bass_utils import run_bass_kernel_spmd

B, IC, OC, KS, W = 32, 64, 64, 3, 2048
NCORES = 8
OWC = W // NCORES      # 256 positions per core
NQ = OWC // 2          # 128 position-pairs per core
GRP = 16               # pairs per PSUM bank / evacuation group
NGRP = NQ // GRP       # 8 groups
DT = mybir.dt.float32
BF = mybir.dt.bfloat16
BF_NP = ml_dtypes.bfloat16

# weight slice schedule (pairs): small first so the PE starts quickly,
# small again at the end so the last compute+store tail is short
SLICES = [(0, 4), (4, 12), (16, 32), (48, 48), (96, 16), (112, 16)]
# xpp column splits (col c feeds pairs q with 2q or 2q+2 == c)
XSPLITS = [(0, 34), (34, 96), (130, 127)]

_compiled_nc = None


def _build_nc():
    nc = bacc.Bacc("TRN2", monotonic_sem_count=0, enable_partition_id=False)

    # Weight layout: SBUF tiles are pair-major [128, L, 256]; the stationary
    # for pair i is the fully CONTIGUOUS 128-col AP wt[:, i, 0:128] (S1) /
    # [:, i, 128:256] (S2), which keeps FWL (fast weight load) on the PE.
    # The two structural zero blocks ARE shipped from DRAM (+2.1MB/core):
    # HW probing showed 128-partition fully-contiguous transfers run at
    # ~350-440GB/s while 64-partition ones (needed by any zero-skipping
    # split) crawl at ~223GB/s, and on-chip repack via vector/gpsimd
    # tensor_copy measured ~30x too slow for strided 64-el runs.
    # wfull row j<64  = [tap0@p | 0      | tap2@p | tap1@p']  (x cols p,p+2)
    # wfull row j>=64 = [tap1@p | tap0@p'| 0      | tap2@p']  (x cols p+1,p+3)
    wfull_d = nc.dram_tensor("wfull", [128, NQ * 256], BF, kind="ExternalInput")
    xpp_d = nc.dram_tensor("xpp", [128, OWC + 1, B], BF, kind="ExternalInput")
    bias_d = nc.dram_tensor("biasq", [128, NQ, 1], DT, kind="ExternalInput")
    out_d = nc.dram_tensor("out", [128, NQ, B], BF, kind="ExternalOutput")

    with tile.TileContext(nc) as tc:
        with (
            tc.tile_pool(name="w", bufs=1) as wpool,
            tc.tile_pool(name="x", bufs=1) as xpool,
            tc.tile_pool(name="ps", bufs=1, space="PSUM") as pspool,
        ):
            xpp = xpool.tile([128, OWC + 1, B], BF, tag="xpp", name="xpp")
            biast = xpool.tile([128, NQ, 1], DT, tag="biast", name="biast")
            # one persistent stationary tile per slice (no buffer reuse;
            # ~64KB per partition total), pair-major layout [128, L, 256]
            wabs = [
                wpool.tile([128, L, 256], BF, tag=f"wab{si}", name=f"wab{si}")
                for si, (q0, L) in enumerate(SLICES)
            ]

            pss = [
                pspool.tile([128, GRP, B], DT, tag=f"ps{i}", name=f"ps{i}")
                for i in range(4)
            ]

            def xpp_part(eng, xi):
                c0, cl = XSPLITS[xi]
                eng.dma_start(
                    out=xpp[:, c0 : c0 + cl, :], in_=xpp_d[:, c0 : c0 + cl, :]
                )

            def load_slice(si):
                q0, L = SLICES[si]
                wt = wabs[si]
                fl = slice(256 * q0, 256 * (q0 + L))
                nc.sync.dma_start(out=wt[:], in_=wfull_d[:, fl])

            # Queue plan: ONE fat sequential stream on the sync queue
            # (concurrent queues split the ~430GB/s single-queue rate and
            # small packets on a second queue disproportionately steal
            # engine time).  Only the tiny bias rides the scalar queue, plus
            # one fat out store at the very end.
            load_slice(0)
            xpp_part(nc.sync, 0)
            nc.scalar.dma_start(out=biast[:], in_=bias_d[:])
            load_slice(1)
            xpp_part(nc.sync, 1)
            load_slice(2)
            xpp_part(nc.sync, 2)
            load_slice(3)
            load_slice(4)
            load_slice(5)

            # single output staging tile (8KB/partition): evacuations land
            # here and ONE fat DMA stores everything at the end
            obs = xpool.tile([128, NQ, B], BF, tag="obs", name="obs")
            for si, (q0, L) in enumerate(SLICES):
                wt = wabs[si]
                for i in range(L):
                    q = q0 + i
                    g = q // GRP
                    ps = pss[g % 4]
                    slot = q % GRP
                    p = 2 * q
                    nc.tensor.matmul(
                        ps[:, slot, :],
                        wt[:, i, 0:128],
                        xpp[:, p, :],
                        start=True,
                        stop=False,
                    )
                    nc.tensor.matmul(
                        ps[:, slot, :],
                        wt[:, i, 128:256],
                        xpp[:, p + 2, :],
                        start=False,
                        stop=True,
                    )
                    if slot == GRP - 1:
                        # evacuate the finished bank: out = psum + bias
                        # (bias broadcast over the 32-batch inner dim)
                        ps_ap, bias_ap = broadcast_tensor_aps(
                            ps[:, :, :], biast[:, g * GRP : (g + 1) * GRP, 0:1]
                        )
                        nc.vector.scalar_tensor_tensor(
                            out=obs[:, g * GRP : (g + 1) * GRP, :],
                            in0=ps_ap,
                            scalar=0.0,
                            in1=bias_ap,
                            op0=mybir.AluOpType.bypass,
                            op1=mybir.AluOpType.add,
                        )
                        # tail stores split across BOTH queues so they run
                        # concurrently: groups 0-3 on sync (fires the moment
                        # the weight stream drains), groups 4-6 and 7 on the
                        # otherwise-idle scalar queue as their evacs finish
                        if g == 3:
                            nc.sync.dma_start(
                                out=out_d[:, 0 : 4 * GRP, :],
                                in_=obs[:, 0 : 4 * GRP, :],
                            )
                        if g == NGRP - 2:
                            nc.scalar.dma_start(
                                out=out_d[:, 4 * GRP : (g + 1) * GRP, :],
                                in_=obs[:, 4 * GRP : (g + 1) * GRP, :],
                            )
            nc.scalar.dma_start(
                out=out_d[:, (NGRP - 1) * GRP :, :], in_=obs[:, (NGRP - 1) * GRP :, :]
            )

    nc.compile()
    return nc


def _get_nc():
    global _compiled_nc
    if _compiled_nc is None:
        _compiled_nc = _build_nc()
    return _compiled_nc


def shard_inputs(x, weights, bias):
    x = np.asarray(x, dtype=np.float32)
    weights = np.asarray(weights, dtype=np.float32)
    bias = np.asarray(bias, dtype=np.float32)

    xp = np.pad(x, ((0, 0), (0, 0), (1, 1)))  # (B, IC, W+2)
    xpT = np.ascontiguousarray(xp.transpose(1, 2, 0))  # (IC, W+2, B)

    in_maps = []
    for c in range(NCORES):
        ws = c * OWC
        xc = xpT[:, ws : ws + OWC + 2, :]  # (64, 258, 32)
        xpp = np.concatenate(
            [xc[:, 0 : OWC + 1, :], xc[:, 1 : OWC + 2, :]], axis=0
        )  # (128, 257, 32)

        Wc = weights[ws : ws + OWC]  # (256, OC, IC, 3)
        We = Wc[0::2].transpose(3, 2, 0, 1)  # (3, IC, NQ, OC) even positions
        Wo = Wc[1::2].transpose(3, 2, 0, 1)  # odd positions
        z = np.zeros_like(We[0])
        # full 256-col stationaries incl. zero blocks (see _build_nc)
        wtop = np.concatenate([We[0], z, We[2], Wo[1]], axis=2)  # (64, NQ, 256)
        wbot = np.concatenate([We[1], Wo[0], z, Wo[2]], axis=2)
        wfull = np.concatenate([wtop, wbot], axis=0)  # (128, NQ, 256)

        def sliced_flat(arr):
            # [P, NQ, C] -> [P, NQ*C], slice-major, pair-major within a
            # slice, matching the [128, L, 256] SBUF tile layout
            parts = [
                arr[:, q0 : q0 + L, :].reshape(arr.shape[0], -1) for q0, L in SLICES
            ]
            return np.concatenate(parts, axis=1)

        bc = bias[:, ws : ws + OWC]  # (64, 256)
        biasq = np.concatenate([bc[:, 0::2], bc[:, 1::2]], axis=0)  # (128, NQ)

        in_maps.append(
            {
                "wfull": np.ascontiguousarray(sliced_flat(wfull)).astype(BF_NP),
                "xpp": np.ascontiguousarray(xpp).astype(BF_NP),
                "biasq": np.ascontiguousarray(biasq[:, :, None]),
            }
        )
    return in_maps


def unshard_output(results):
    out = np.empty((B, OC, W), np.float32)
    for c in range(NCORES):
        ws = c * OWC
        r = np.asarray(results[c]["out"], dtype=np.float32)  # (128, NQ, B)
        # r[s*64+oc, q, b] -> out[b, oc, ws + 2q + s]
        rr = r.reshape(2, OC, NQ, B).transpose(3, 1, 2, 0)  # (B, OC, NQ, 2)
        out[:, :, ws : ws + OWC] = rr.reshape(B, OC, OWC)
    return out


def run_sharded(x, weights, bias, trace=False):
    nc = _get_nc()
    in_maps = shard_inputs(x, weights, bias)
    res = run_bass_kernel_spmd(nc, in_maps, list(range(NCORES)), trace=trace)
    return unshard_output(res.results), res


def kernel(x, weights, bias):
    out, _ = run_sharded(x, weights, bias)
    return out
